# revision 62
# baseline (speedup 1.0000x reference)
"""Trainium2 Bass kernel for nn_CapsuleLayer (dynamic routing, 3 iterations).

Full problem:
  x:  [64, 2048, 16] f32;  route_weights: [32, 2048, 16, 32] f32
  priors[b,c,n,o] = sum_i x[b,n,i] * rw[c,n,i,o]
  3 rounds of routing-by-agreement (softmax over n=2048); output [64, 32, 32].

Sharding: capsule-parallel over 8 cores (4 caps/core), batch replicated.

Per-core dataflow (cl = local cap 0..3), heavy contractions on the PE:
  phase A   s1[b,(cl,o)]   = sum_{n,i} x*rw          K=(n,i) dense, uniform probs
  e'-pass   e'[(i,b), n]   = sum_o out[b,cl,o]*rw    block-diag weights, K=(2i,32o)
            xe             = e' * x                  DVE/Pool mul (psum -> sbuf fp16)
            dB[b, n]       = sum_i xe                PE selector OR DVE/Pool halving tree
  softmax   w~ = exp(B - max); Z from ACT accum_out
  s~-pass   wbt = transpose(w~) (PE), staged to SBUF (Pool)
            xw  = xT * wbt                           DVE/Pool fp16
            s~[(cl,o),b] = sum_{n,i} xw * rw         K=(n,i), lhsT=rw4 col-slices
  squash    alpha = sqrt(sq)/(den+sq); out = alpha * s~   (den = N^2 or Z^2)

Engine budget per iteration (cost-model): PE 63us, DVE ~55us, ACT ~47us,
Pool ~55us; everything SBUF-resident except rwt (streamed, 8.4MB/iter).
"""

import sys

sys.path.insert(0, "/opt/trn_rl_repo")

import numpy as np

import concourse.bass as bass
import concourse.bacc as bacc
import concourse.tile as tile
import concourse.mybir as mybir
from concourse.bass_utils import run_bass_kernel_spmd

F16 = mybir.dt.float16
F32 = mybir.dt.float32
AX = mybir.AxisListType
ALU = mybir.AluOpType
ACTF = mybir.ActivationFunctionType

B, C, N, ID, OD = 64, 32, 2048, 16, 32
NCORES = 8
CL = C // NCORES          # 4 local caps per core
G = N // 128              # 16 n-blocks of 128

f16 = np.float16

CL_ORDER = [0, 2, 1, 3]   # finish softmax plane 0 (cl 0,2) before plane 1


def _bcast_alpha(alpha_ap):
    """[64, CL] -> [64, CL, OD] stride-0 broadcast view."""
    return alpha_ap.unsqueeze(2).broadcast_to((B, CL, OD))


def _build():
    nc = bacc.Bacc("TRN2")

    xT_d = nc.dram_tensor("xT", [128, G, ID, B], F16, kind="ExternalInput")
    rw4_d = nc.dram_tensor("rw4", [128, G, ID, 128], F16, kind="ExternalInput")
    rwt_d = nc.dram_tensor("rwt", [CL, 4, 128, N], F16, kind="ExternalInput")
    x2e_d = nc.dram_tensor("x2e", [128, 8, N], F16, kind="ExternalInput")
    out_d = nc.dram_tensor("out", [B, CL * OD], F32, kind="ExternalOutput")

    s2sel_d = nc.inline_tensor(np.tile(np.eye(B, dtype=f16), (2, 1)), name="s2sel")
    id64_d = nc.inline_tensor(np.eye(64, dtype=np.float32), name="id64")
    id128_d = nc.inline_tensor(np.eye(128, dtype=np.float32), name="id128")
    e4_d = nc.inline_tensor(
        np.repeat(np.eye(4, dtype=np.float32), 32, axis=1), name="e4"
    )
    onesblk_d = nc.inline_tensor(
        np.kron(np.eye(4, dtype=np.float32), np.ones((32, 1), dtype=np.float32)),
        name="onesblk"
    )
    s2sel32_d = nc.inline_tensor(
        np.tile(np.eye(B, dtype=np.float32), (2, 1)), name="s2sel32"
    )

    with tile.TileContext(nc) as tc:
        with (
            tc.tile_pool(name="res", bufs=1) as res,
            tc.tile_pool(name="stream", bufs=2) as stream,
            tc.tile_pool(name="small", bufs=2) as small,
            tc.tile_pool(name="pbig", bufs=2, space="PSUM") as pbig,
            tc.tile_pool(name="pacc", bufs=2, space="PSUM") as pacc,
            tc.tile_pool(name="psml", bufs=1, space="PSUM") as psml,
        ):
            xT = res.tile([128, G, ID, B], F16)
            s2sel = res.tile([128, B], F16)
            rw4 = res.tile([128, G, ID, 128], F16)
            x2e = res.tile([128, 8, N], F16)
            id64 = res.tile([64, 64], F32)
            id128 = res.tile([128, 128], F32)
            e4c = res.tile([4, 128], F32)
            onesblk = res.tile([128, 4], F32)
            s2sel32 = res.tile([128, B], F32)
            Lz = res.tile([128, 4], F32)
            Bst = res.tile([128, 2, N], F16)
            dbts = [res.tile([128, 4, 2, B], F16, name=f"dbt{u}")
                    for u in range(4)]
            wb = res.tile([128, 2, N], F16)
            Z = res.tile([128, 2], F32)
            mxp = res.tile([128, 2, 4], F32)
            outwBD = res.tile([128, CL, 128], F16)

            # phase A streams split across the 3 DMA-capable queues
            # (SP/ACT hwdge + Pool swdge) so loads land ~3x faster; phase-A
            # matmuls chase interleaved chunk arrival (see GA below).
            for g in range(0, 12, 2):
                q = nc.sync if g < 6 else nc.scalar
                q.dma_start(out=xT[:, g:g + 2], in_=xT_d[:, g:g + 2])
                q.dma_start(out=rw4[:, g:g + 2], in_=rw4_d[:, g:g + 2])
            for g in range(12, G, 2):
                nc.gpsimd.dma_start(out=xT[:, g:g + 2], in_=xT_d[:, g:g + 2])
                nc.gpsimd.dma_start(out=rw4[:, g:g + 2], in_=rw4_d[:, g:g + 2])
            nc.gpsimd.dma_start(out=s2sel, in_=s2sel_d[:, :])
            nc.gpsimd.dma_start(out=id64, in_=id64_d[:, :])
            nc.gpsimd.dma_start(out=id128, in_=id128_d[:, :])
            nc.gpsimd.dma_start(out=e4c, in_=e4_d[:, :])
            nc.gpsimd.dma_start(out=onesblk, in_=onesblk_d[:, :])
            nc.gpsimd.dma_start(out=s2sel32, in_=s2sel32_d[:, :])
            nc.gpsimd.memset(Lz, 0.0)
            # every e' tile reads all 8 x2e planes (plane pair = i-pair);
            # must be fully resident before the first xe-mul
            nc.sync.dma_start(out=x2e[:, 0:4], in_=x2e_d[:, 0:4])
            nc.scalar.dma_start(out=x2e[:, 4:8], in_=x2e_d[:, 4:8])
            nc.gpsimd.memset(Bst, 0.0)
            nc.gpsimd.memset(outwBD, 0.0)

            # ---------------- phase A: s1 ----------------
            # queue chunk arrival order: SP g0..g5 | ACT g6..g11 | Pool
            # g12..g15 stream concurrently; interleave matmul emission so
            # the PE chases all three queues instead of just SP.
            GA = [0, 6, 12, 1, 7, 13, 2, 8, 14, 3, 9, 15, 4, 10, 5, 11]
            ps1 = pacc.tile([B, 128], F32, tag="acc", bufs=1)
            for gi, g in enumerate(GA):
                for i in range(ID):
                    nc.tensor.matmul(
                        ps1,
                        xT[:, g, i],
                        rw4[:, g, i],
                        start=(gi == 0 and i == 0),
                        stop=(gi == G - 1 and i == ID - 1),
                    )

            def compute_alpha(sq_ap, den_const=None, use_z=False, Zg=None,
                              w=CL):
                # alpha = sqrt(sq) / (den + sq); den = const or Z^2
                alpha = small.tile([B, w], F32, tag=f"alpha{w}", bufs=3)
                rs = small.tile([B, w], F32, tag=f"rs{w}")
                nc.scalar.activation(rs, sq_ap, ACTF.Sqrt)
                den = small.tile([B, w], F32, tag=f"den{w}")
                if use_z:
                    z2 = small.tile([B, w], F32, tag=f"z2{w}")
                    nc.vector.tensor_mul(z2, Zg, Zg)
                    nc.vector.tensor_add(den, z2, sq_ap)
                else:
                    nc.vector.tensor_scalar_add(den, sq_ap, float(den_const))
                rec = small.tile([B, w], F32, tag=f"rec{w}")
                nc.vector.reciprocal(rec, den)
                nc.vector.tensor_mul(alpha, rs, rec)
                return alpha

            def build_outw4(outT_sb):
                # outT_sb [128 (cl,o), 64 b] -> outwBD [128, cl, 128] block-diag:
                # rows (rh, isub', o), cols (isub, b); diag blocks = outT[cl-block]
                for cl in range(CL):
                    for rh in range(2):          # row half (for rhs base 0/64)
                        for isub in range(2):    # diagonal block
                            nc.gpsimd.tensor_copy(
                                outwBD[64 * rh + 32 * isub:64 * rh + 32 * isub + 32,
                                       cl, 64 * isub:64 * isub + 64],
                                outT_sb[32 * cl:32 * cl + 32],
                            )

            # iteration-1 squash (uniform probs; divide-by-N folded into alpha)
            sq3 = small.tile([B, CL, OD], F32, tag="sq3")
            nc.scalar.activation(
                sq3, ps1.rearrange("b (c o) -> b c o", c=CL), ACTF.Square
            )
            sqS = small.tile([B, CL], F32, tag="sqS")
            nc.vector.tensor_reduce(sqS, sq3, axis=AX.X, op=ALU.add)
            alpha = compute_alpha(sqS, den_const=float(N) * float(N))
            out1 = small.tile([B, CL, OD], F32, tag="outbco")
            nc.vector.tensor_mul(
                out1, ps1.rearrange("b (c o) -> b c o", c=CL), _bcast_alpha(alpha)
            )
            ptr = psml.tile([128, 64], F32, tag="psm")
            nc.tensor.transpose(ptr, out1.rearrange("b c o -> b (c o)"), id64)
            outT1 = small.tile([128, B], F32, tag="outT")
            nc.scalar.copy(outT1, ptr)
            build_outw4(outT1)

            # ---------------- iterations 2, 3 ----------------
            def emit_softmax(plane):
                # w~ = exp(B - max); Z = sum w~ via ACT accumulator
                # (max is combined from per-u partials computed as dB lands)
                mx = small.tile([128, 1], F32, tag="mx")
                nc.vector.tensor_reduce(mx, mxp[:, plane], axis=AX.X, op=ALU.max)
                negm = small.tile([128, 1], F32, tag="negm")
                nc.vector.tensor_scalar_mul(negm, mx, -1.0)
                nc.scalar.activation(
                    wb[:, plane], Bst[:, plane], ACTF.Exp,
                    bias=negm, scale=1.0,
                    accum_out=Z[:, plane:plane + 1],
                )

            def emit_wtrans(plane, wbT):
                # w~ [b(plane-packed), n] -> wbT [n, (half, b)] via DMA xbar
                for g in range(G):
                    nc.sync.dma_start(
                        out=wbT[:, g].rearrange("p h b -> p (h b)"),
                        in_=wb[:, plane, 128 * g:128 * g + 128],
                        transpose=True,
                    )

            for it in range(2, 4):
                final = it == 3
                sel_q = []   # (cl, u, xe4) awaiting transposed PE reduce

                def service_sel():
                    # dB^T = sum_i xe: 32 matmuls with xe as stationary lhsT
                    # ([128,128] chunk) and s2sel moving -> 64 rows/step.
                    # Result [n-sub, b] is converted to fp16 and, once both
                    # capsules of the plane have landed in the shared dbt
                    # tile, DMA-transposed back to [b(plane-packed), n] and
                    # folded into Bst on DVE.
                    cl, u, sxe = sel_q.pop(0)
                    plane, half = cl % 2, cl // 2
                    pdbT = pacc.tile([128, 4, B], F32, tag="pdbT", bufs=2)
                    for c in range(4):
                        for k in range(8):
                            nc.tensor.matmul(
                                pdbT[:, c],
                                sxe[:, k, 128 * c:128 * c + 128],
                                s2sel,
                                start=(k == 0),
                                stop=(k == 7),
                            )
                    dbt = dbts[u]
                    nc.scalar.copy(dbt[:, :, half], pdbT)
                    if half == 1:
                        trt = stream.tile([128, 4, 128], F16, tag="trt", bufs=2)
                        for c in range(4):
                            nc.sync.dma_start(
                                out=trt[:, c],
                                in_=dbt[:, c].rearrange("p h b -> p (h b)"),
                                transpose=True,
                            )
                        bslc = Bst[:, plane, 512 * u:512 * u + 512]
                        nc.vector.tensor_add(
                            bslc, bslc, trt.rearrange("p c n -> p (c n)")
                        )
                        nc.vector.tensor_reduce(
                            mxp[:, plane, u:u + 1], bslc, axis=AX.X, op=ALU.max
                        )

                def emit_eprime(cl, stage_x2e):
                    for u in range(4):
                        nslc = slice(512 * u, 512 * u + 512)
                        rt = stream.tile([128, 4, 512], F16, tag="rwt", bufs=2)
                        nc.sync.dma_start(
                            out=rt.rearrange("p a b -> p (a b)"), in_=rwt_d[cl, u]
                        )
                        xe4 = stream.tile([128, 8, 512], F16, tag="xe4", bufs=2)
                        for ib in range(4):
                            pe = pbig.tile([128, 2, 512], F32, tag="pb", bufs=2)
                            for half in range(2):
                                nc.tensor.matmul(
                                    pe[:, half],
                                    outwBD[64 * half:64 * half + 64, cl],
                                    rt[64 * half:64 * half + 64, ib],
                                    start=True,
                                    stop=True,
                                )
                            x2s = x2e[:, 2 * ib:2 * ib + 2, nslc]
                            if ib in (0, 3):
                                # DVE: fused convert+mul straight from PSUM
                                # (GPSIMD has no PSUM port)
                                nc.vector.tensor_mul(
                                    xe4[:, 2 * ib:2 * ib + 2], pe, x2s
                                )
                            else:
                                # ACT convert; DVE (ib0) or Pool (ib1/2) mul
                                eb = stream.tile(
                                    [128, 2, 512], F16, tag="eb", bufs=2
                                )
                                nc.scalar.copy(eb, pe)
                                nc.gpsimd.tensor_mul(
                                    xe4[:, 2 * ib:2 * ib + 2], eb, x2s
                                )
                        # transposed dB reduce deferred one u so the PE never
                        # waits on the xe conversion chain
                        if sel_q:
                            service_sel()
                        sel_q.append((cl, u, xe4))

                def emit_stilde(cl, pst, wbT, ndve=6):
                    half = cl // 2
                    pos = cl
                    for g2 in range(G // 2):
                        xw2 = stream.tile([128, 2, ID, B], F16, tag="xw4", bufs=3)
                        xw_eng = nc.vector if g2 < ndve else nc.gpsimd
                        xw_eng.tensor_mul(
                            xw2,
                            xT[:, 2 * g2:2 * g2 + 2],
                            wbT[:, 2 * g2:2 * g2 + 2, half]
                            .unsqueeze(2).broadcast_to((128, 2, ID, B)),
                        )
                        for gg in range(2):
                            g = 2 * g2 + gg
                            for i in range(ID):
                                nc.tensor.matmul(
                                    pst[32 * pos:32 * pos + 32],
                                    rw4[:, g, i, 32 * cl:32 * cl + 32],
                                    xw2[:, gg, i],
                                    start=(g2 == 0 and gg == 0 and i == 0),
                                    stop=(g2 == G // 2 - 1 and gg == 1
                                          and i == ID - 1),
                                    tile_position=(0, 32 * pos),
                                )

                def flush_sel():
                    while sel_q:
                        service_sel()

                # interleave: s~ of finished planes runs between e' capsules so
                # PE-heavy and DVE/ACT/Pool-heavy stretches overlap
                pst = pacc.tile([128, B], F32, tag="acc", bufs=1)
                emit_eprime(0, stage_x2e=(it == 2))
                emit_eprime(2, stage_x2e=False)
                flush_sel()
                emit_softmax(0)
                wbT0 = small.tile([128, G, 2, B], F16, tag="wbT", bufs=2)
                emit_wtrans(0, wbT0)
                emit_eprime(1, stage_x2e=False)
                emit_stilde(0, pst, wbT0)
                emit_eprime(3, stage_x2e=False)
                flush_sel()
                emit_stilde(2, pst, wbT0)
                emit_softmax(1)
                wbT1 = small.tile([128, G, 2, B], F16, tag="wbT", bufs=2)
                emit_wtrans(1, wbT1)
                # hoist Z-dependent alpha terms off the squash critical path:
                # ZT[cl, b] = Z[64*(cl//2)+b, cl%2] via masked fp32 matmul
                psm = psml.tile([128, 128], F32, tag="psm")
                for cl in range(CL):
                    h, pl = cl // 2, cl % 2
                    nc.gpsimd.tensor_copy(
                        Lz[64 * h:64 * h + 64, cl:cl + 1],
                        Z[64 * h:64 * h + 64, pl:pl + 1],
                    )
                pZT = psm[0:4, 64:128]
                nc.tensor.matmul(pZT, Lz, s2sel32, start=True, stop=True)
                sqv = small.tile([4, 4, B], F32, tag="sqv", bufs=1)
                ZTs, z2T, rsT, denT = (sqv[:, j] for j in range(4))
                nc.scalar.copy(ZTs, pZT)
                nc.vector.tensor_mul(z2T, ZTs, ZTs)
                emit_stilde(1, pst, wbT1)
                emit_stilde(3, pst, wbT1)
                # --- squash, alpha computed in transposed [cl, b] layout ---
                sqT = small.tile([128, B], F32, tag="sqT")
                nc.scalar.activation(sqT, pst, ACTF.Square)
                stT = small.tile([128, B], F32, tag="stT")
                nc.scalar.copy(stT, pst)
                psqT = psm[0:4, 0:B]
                nc.tensor.matmul(psqT, onesblk, sqT, start=True, stop=True)
                nc.scalar.activation(rsT, psqT, ACTF.Sqrt)
                nc.vector.tensor_add(denT, z2T, psqT)
                recT = ZTs      # ZTs dead after z2T; reuse its slot
                nc.vector.reciprocal(recT, denT)
                alphaT = z2T    # z2T dead after denT; reuse
                nc.vector.tensor_mul(alphaT, rsT, recT)
                pa32 = psm[0:128, 0:B]
                nc.tensor.matmul(pa32, e4c, alphaT, start=True, stop=True)
                if not final:
                    outT = small.tile([128, B], F16, tag="outT")
                    nc.vector.tensor_mul(outT, stT, pa32)
                    build_outw4(outT)
                else:
                    outT32 = small.tile([128, B], F32, tag="outT32")
                    nc.vector.tensor_mul(outT32, stT, pa32)
                    psof = psm[0:64, :]
                    nc.tensor.transpose(psof, outT32, id128)
                    outf = small.tile([B, 128], F16, tag="outf")
                    nc.scalar.copy(outf, psof)
                    nc.gpsimd.dma_start(out=out_d[:, :], in_=outf)

    return nc


_NC_CACHE = {}


def _get_nc():
    if "nc" not in _NC_CACHE:
        nc = _build()
        nc.finalize()
        _NC_CACHE["nc"] = nc
    return _NC_CACHE["nc"]


def prep_inputs(x, rw):
    """Host-side layout prep; returns per-core input maps."""
    x = np.asarray(x, dtype=np.float32)
    rw = np.asarray(rw, dtype=np.float32)

    # xT [128, G, ID, B]: (p, g, i, b) = x[b, 128g+p, i]
    xT_h = np.ascontiguousarray(
        x.reshape(B, G, 128, ID).transpose(2, 1, 3, 0).astype(f16)
    )
    # x2e [128, 8, N]: q<64 -> x[q, n, 2k]; q>=64 -> x[q-64, n, 2k+1]
    x2e_h = np.empty((128, 8, N), dtype=f16)
    xt = x.transpose(2, 0, 1).astype(f16)  # [i, b, n]
    for k in range(8):
        x2e_h[:64, k] = xt[2 * k]
        x2e_h[64:, k] = xt[2 * k + 1]

    in_maps = []
    for core in range(NCORES):
        rws = rw[CL * core: CL * core + CL]  # [4, N, ID, OD]
        rw4_h = np.ascontiguousarray(
            rws.reshape(CL, G, 128, ID, OD).transpose(2, 1, 3, 0, 4)
            .reshape(128, G, ID, CL * OD).astype(f16)
        )
        # rwt [cl, u, 32r+o, (ib, n')] = rw[cl, 512u+n', 4ib+r, o]
        rwt_h = np.ascontiguousarray(
            rws.reshape(CL, N, 4, 4, OD).transpose(0, 2, 3, 4, 1)
            .reshape(CL, 4, 128, 4, 512).transpose(0, 3, 2, 1, 4)
            .reshape(CL, 4, 128, N).astype(f16)
        )
        in_maps.append({"xT": xT_h, "rw4": rw4_h, "rwt": rwt_h, "x2e": x2e_h})
    return in_maps


def kernel(x, route_weights, ncores=NCORES, trace=False):
    in_maps = prep_inputs(x, route_weights)
    nc = _get_nc()
    res = run_bass_kernel_spmd(nc, in_maps[:ncores], core_ids=list(range(ncores)))
    outs = [r["out"].reshape(B, CL, OD) for r in res.results]
    return np.concatenate(outs, axis=1).astype(np.float32)


if __name__ == "__main__":
    rng = np.random.default_rng(0)
    x = rng.standard_normal((B, N, ID), dtype=np.float32)
    rw = rng.standard_normal((C, N, ID, OD), dtype=np.float32)
    out = kernel(x, rw)
    print(out.shape, out.dtype, float(np.abs(out).mean()))

